# revision 19
# baseline (speedup 1.0000x reference)
"""BGRL (nn_BGRL) Trainium2 8-core Bass kernel.

Math restructuring: segment_sum is linear and h = x@W, so
  gcn(x, W) = segsum(val * (x@W)[col], row) + b = segsum(val * x[col], row) @ W + b.
With P = segsum(val * x[col], row) and Q = segsum(val * perb[col], row):
  gcn(x1, W)  = P@W + b          (online_x)
  gcn(x2, W)  = (P+Q)@W + b      (enc_x2 = online_y)
  gcn(x1, Wt) = P@Wt + bt        (target_y)
  gcn(x2, Wt) = (P+Q)@Wt + bt    (target_x)
So only TWO aggregations over raw [x|perb] (1KB/edge gather) are needed.

Sharding: nodes row-partitioned across 8 cores (6250 rows each); edges
partitioned by destination row so the segment-sum is core-local; the gather
table G=[x|perb] (50 MB) is replicated in each core's HBM (staged pre-kernel,
so no all-gather inside the kernel).

Device algorithm per core:
  Phase 1 (per 128-dest-row window, 49 windows):
    dma_gather rows of G for this window's edges (sorted by dest, padded to
    128-edge chunks; G split into two <32768-row halves for int16 indices);
    selection matrix S[e,d] = (iota[d]==dest_local[e])*val[e] built by one
    fused DVE op per chunk; PSUM-accumulated matmul S^T @ msg gives
    agg = [P|Q] rows for the window; PE-transpose into aggPT/aggQT (feature-
    on-partition layout, which phase 2 needs for per-feature BN broadcast).
  Phase 2 (per 512-row block, 13 blocks, all in transposed layout):
    4 GCN matmuls from aggPT/aggQT + embed output; predictor pass A for BN
    statistics; tiny AllReduce (2KB) of stats; BN-apply + PReLU + predictor
    pass B; BYOL cosine terms via ones-matmul column sums; scalar AllReduce
    of the loss partial.

Outputs: embedt [128, 6250] per core (transposed embed shard) and the loss
scalar; host transposes/concats shards and returns (embed [50000,128], loss).
"""
import math
import numpy as np
from contextlib import ExitStack

import concourse.bass as bass
import concourse.tile as tile
from concourse import bacc, mybir
from concourse.bass_utils import run_bass_kernel_spmd
from concourse.masks import make_identity

N, D, E = 50000, 128, 800000
NCORES = 8
RPC = N // NCORES            # 6250 rows per core
NW = (RPC + 127) // 128      # 49 windows of 128 dest rows
HALF = N // 2                # int16 gather-table split
BLK = 512
NB = (RPC + BLK - 1) // BLK  # 13 phase-2 blocks
BN_EPS = 1e-5
GD = 2 * D                   # 256 features in G = [x | perb]
GMAX = 8                     # max 128-chunks per dma_gather call

F32 = mybir.dt.float32

# test harness hooks (default off; the grading path never sets these)
TRACE = False
TRACE_KW: dict = {}
LAST: dict = {}
PHASES = 2  # debug: 1 = aggregation only (dump aggPT, skip predictor+collectives)


def _prep_edges(edge_row, edge_col, edge_val):
    """Partition/sort/pad edges. Returns (nch [NW,2] shared chunk counts,
    per-core dicts with eidx/dl/val arrays)."""
    core = edge_row // RPC
    local = edge_row - core * RPC
    win = local // 128
    dl = (local % 128).astype(np.float32)
    hf = (edge_col >= HALF).astype(np.int64)
    sl = (edge_col - hf * HALF).astype(np.int64)

    key = ((core * NW + win) * 2 + hf).astype(np.int64)
    order = np.argsort(key, kind="stable")
    key_s = key[order]
    sl_s = sl[order]
    dl_s = dl[order]
    va_s = edge_val[order].astype(np.float32)

    counts = np.bincount(key_s, minlength=NCORES * NW * 2).reshape(NCORES, NW, 2)
    cmax = counts.max(axis=0)                            # [NW, 2]
    nch = np.maximum(np.ceil(cmax / 128.0), 1).astype(np.int64)
    # gathered rows per segment: exact max across cores, 16-aligned (the
    # trailing lanes of a partial chunk are never gathered; their dl/val are
    # zero so the matmul ignores whatever the recycled tile holds there)
    nidx = (np.ceil(cmax / 16.0) * 16).astype(np.int64)  # [NW, 2]
    totch = int(nch.sum())
    totidx = int(nidx.sum())

    seg_start = np.zeros(NCORES * NW * 2 + 1, dtype=np.int64)
    np.cumsum(counts.reshape(-1), out=seg_start[1:])

    cores = []
    for c in range(NCORES):
        flat_sl = np.zeros(totidx, dtype=np.int16)
        flat_dl = np.zeros(totch * 128, dtype=np.float32)
        flat_va = np.zeros(totch * 128, dtype=np.float32)
        ioff = 0
        off = 0
        for w in range(NW):
            for h in range(2):
                k = (c * NW + w) * 2 + h
                s0, s1 = seg_start[k], seg_start[k + 1]
                n = s1 - s0
                flat_sl[ioff : ioff + n] = sl_s[s0:s1]
                flat_dl[off : off + n] = dl_s[s0:s1]
                flat_va[off : off + n] = va_s[s0:s1]
                ioff += int(nidx[w, h])
                off += int(nch[w, h]) * 128
        assert off == totch * 128 and ioff == totidx
        # wrapped int16 idx layout, replicated across the 8 Q7 stripes
        w16 = flat_sl.reshape(-1, 16).T.copy()          # [16, totidx//16]
        eidx = np.tile(w16, (8, 1))                     # [128, totidx//16]
        dlm = flat_dl.reshape(totch, 128).T.copy()      # [128, totch]
        vam = flat_va.reshape(totch, 128).T.copy()
        cores.append(dict(eidx=eidx, dl=dlm, val=vam))
    return (nch, nidx), totch, totidx, cores


def _build(nchidx, totch, totidx):
    nch, nidx = nchidx
    nc = bacc.Bacc("TRN2", target_bir_lowering=False, debug=False,
                   num_devices=NCORES)
    op = mybir.AluOpType

    g0 = nc.dram_tensor("g0", [HALF, GD], MMDT, kind="ExternalInput")
    g1 = nc.dram_tensor("g1", [N - HALF, GD], MMDT, kind="ExternalInput")
    eidx = nc.dram_tensor("eidx", [128, totidx // 16], mybir.dt.int16,
                          kind="ExternalInput")
    dlv = nc.dram_tensor("dlv", [128, totch], F32, kind="ExternalInput")
    vav = nc.dram_tensor("vav", [128, totch], F32, kind="ExternalInput")
    x2t = nc.dram_tensor("x2t", [128, RPC], F32, kind="ExternalInput")
    wcat = nc.dram_tensor("wcat", [128, 4 * 128], F32, kind="ExternalInput")
    prm = nc.dram_tensor("prm", [128, 8], F32, kind="ExternalInput")
    iot = nc.dram_tensor("iot", [128, 128], F32, kind="ExternalInput")
    embedt = nc.dram_tensor("embedt", [128, RPC], F32, kind="ExternalOutput")
    lossout = nc.dram_tensor("lossout", [1, 8], F32, kind="ExternalOutput")

    with tile.TileContext(nc) as tc, ExitStack() as ctx:
        cst = ctx.enter_context(tc.tile_pool(name="cst", bufs=1))
        gp = ctx.enter_context(tc.tile_pool(name="gp", bufs=2))
        sp = ctx.enter_context(tc.tile_pool(name="sp", bufs=3))
        ap = ctx.enter_context(tc.tile_pool(name="ap", bufs=2))
        bp = ctx.enter_context(tc.tile_pool(name="bp", bufs=2))
        sm = ctx.enter_context(tc.tile_pool(name="sm", bufs=2))
        byp = ctx.enter_context(tc.tile_pool(name="byp", bufs=3))
        ps1 = ctx.enter_context(tc.tile_pool(name="ps1", bufs=2, space="PSUM"))
        pst = ctx.enter_context(tc.tile_pool(name="pst", bufs=1, space="PSUM"))
        ps2 = ctx.enter_context(tc.tile_pool(name="ps2", bufs=2, space="PSUM"))
        psb = ctx.enter_context(tc.tile_pool(name="psb", bufs=3, space="PSUM"))
        dram = ctx.enter_context(tc.tile_pool(name="dram", bufs=1, space="DRAM"))

        # ---- resident constants ----
        eidx_t = cst.tile([128, totidx // 16], mybir.dt.int16, tag="eidx")
        nc.sync.dma_start(eidx_t[:], eidx[:])
        dl_t = cst.tile([128, totch], F32, tag="dl")
        nc.sync.dma_start(dl_t[:], dlv[:])
        va_t = cst.tile([128, totch], F32, tag="va")
        nc.sync.dma_start(va_t[:], vav[:])
        iota_t = cst.tile([128, 128], F32, tag="iota")
        nc.sync.dma_start(iota_t[:], iot[:])
        w_t = cst.tile([128, 4 * 128], F32, tag="wcat")
        nc.sync.dma_start(w_t[:], wcat[:])
        prm_t = cst.tile([128, 8], F32, tag="prm")
        nc.sync.dma_start(prm_t[:], prm[:])
        ident = cst.tile([128, 128], F32, tag="ident")
        make_identity(nc, ident[:])

        W_ = w_t[:, 0:128]
        Wt_ = w_t[:, 128:256]
        W1_ = w_t[:, 256:384]
        W2_ = w_t[:, 384:512]
        b_ = prm_t[:, 0:1]
        bt_ = prm_t[:, 1:2]
        b1_ = prm_t[:, 2:3]
        b2_ = prm_t[:, 3:4]
        gam_ = prm_t[:, 4:5]
        bet_ = prm_t[:, 5:6]
        pa_ = prm_t[:, 6:7]
        eps_ = prm_t[:, 7:8]

        ones_t = cst.tile([128, 1], F32, tag="ones")
        nc.vector.memset(ones_t[:], 1.0)
        zeros_t = cst.tile([128, 1], F32, tag="zeros")
        nc.vector.memset(zeros_t[:], 0.0)

        aggPT = cst.tile([128, NW * 128], F32, tag="aggPT")
        aggQT = cst.tile([128, NW * 128], F32, tag="aggQT")

        stats = cst.tile([128, 4], F32, tag="stats")
        nc.vector.memset(stats[:], 0.0)
        dacc = cst.tile([1, BLK], F32, tag="dacc")
        nc.vector.memset(dacc[:], 0.0)
        dacc2 = cst.tile([1, BLK], F32, tag="dacc2")
        nc.vector.memset(dacc2[:], 0.0)

        maxC = int((nch[:, 0] + nch[:, 1]).max())

        # ---- phase 1: aggregation ----
        icol = 0
        gch = 0
        for w in range(NW):
            c0, c1 = int(nch[w, 0]), int(nch[w, 1])
            Cw = c0 + c1
            gath = gp.tile([128, maxC, GD], MMDT, tag="gath")
            if w < 2:
                # first use of each rotating slot: clear so never-gathered
                # lanes of partial chunks hold finite values (0*finite == 0)
                nc.vector.memset(gath[:], 0.0)
            coff = 0
            for hfi, cnt in ((0, c0), (1, c1)):
                n_left = int(nidx[w, hfi])
                if n_left == 0:
                    coff += cnt
                    continue
                src = g0 if hfi == 0 else g1
                # cap each dma_gather at GMAX chunks (descriptor-ring pressure)
                while n_left > 0:
                    n_i = min(GMAX * 128, n_left)
                    nchunk = (n_i + 127) // 128
                    nc.gpsimd.dma_gather(
                        gath[:, coff : coff + nchunk, :],
                        src[:],
                        eidx_t[:, icol : icol + (n_i + 15) // 16],
                        n_i,
                        n_i,
                        GD,
                    )
                    coff += nchunk
                    icol += (n_i + 15) // 16
                    n_left -= n_i
            assert coff <= Cw
            agg_ps = ps1.tile([128, GD], F32)
            for ci in range(Cw):
                s_c = sp.tile([128, 128], F32, tag="s")
                nc.vector.tensor_scalar(
                    out=s_c[:],
                    in0=iota_t[:],
                    scalar1=dl_t[:, gch : gch + 1],
                    scalar2=va_t[:, gch : gch + 1],
                    op0=op.is_equal,
                    op1=op.mult,
                )
                nc.tensor.matmul(
                    out=agg_ps[:],
                    lhsT=s_c[:],
                    rhs=gath[:, ci, :],
                    start=(ci == 0),
                    stop=(ci == Cw - 1),
                )
                gch += 1
            agg_sb = ap.tile([128, GD], F32, tag="agg")
            nc.scalar.copy(agg_sb[:], agg_ps[:])
            for hi, dstT in ((0, aggPT), (1, aggQT)):
                tp = pst.tile([128, 128], F32)
                nc.tensor.transpose(
                    out=tp[:], in_=agg_sb[:, hi * 128 : (hi + 1) * 128],
                    identity=ident[:],
                )
                nc.vector.tensor_copy(
                    out=dstT[:, w * 128 : (w + 1) * 128], in_=tp[:]
                )
        assert gch == totch and icol == totidx // 16, (gch, icol)

        if PHASES == 1:
            nc.sync.dma_start(embedt[:], aggPT[:, :RPC])
            lz = sm.tile([1, 8], F32, tag="lvec")
            nc.vector.memset(lz[:], 0.0)
            nc.sync.dma_start(lossout[:], lz[:])
            return _finish(nc)

        # ---- phase 2A: embed + BN stats ----
        for b in range(NB):
            nb = min(BLK, RPC - b * BLK)
            cs = slice(b * BLK, b * BLK + nb)
            PT = aggPT[:, cs]
            QT = aggQT[:, cs]

            rt = bp.tile([128, BLK], F32, tag="rt")
            nc.vector.tensor_add(rt[:, :nb], PT, QT)

            enc_ps = ps2.tile([128, BLK], F32, tag="mm")
            nc.tensor.matmul(out=enc_ps[:, :nb], lhsT=W_, rhs=rt[:, :nb],
                             start=True, stop=True)
            enc = bp.tile([128, BLK], F32, tag="enc")
            nc.vector.tensor_scalar(out=enc[:, :nb], in0=enc_ps[:, :nb],
                                    scalar1=b_, scalar2=None, op0=op.add)

            x2b = bp.tile([128, BLK], F32, tag="x2b")
            nc.sync.dma_start(x2b[:, :nb], x2t[:, cs])
            emb = bp.tile([128, BLK], F32, tag="emb")
            nc.vector.tensor_add(emb[:, :nb], x2b[:, :nb], enc[:, :nb])
            nc.sync.dma_start(embedt[:, cs], emb[:, :nb])

            onl_ps = ps2.tile([128, BLK], F32, tag="mm")
            nc.tensor.matmul(out=onl_ps[:, :nb], lhsT=W_, rhs=PT,
                             start=True, stop=True)
            onl = bp.tile([128, BLK], F32, tag="onl")
            nc.vector.tensor_scalar(out=onl[:, :nb], in0=onl_ps[:, :nb],
                                    scalar1=b_, scalar2=None, op0=op.add)

            hx_ps = ps2.tile([128, BLK], F32, tag="mm")
            nc.tensor.matmul(out=hx_ps[:, :nb], lhsT=W1_, rhs=onl[:, :nb],
                             start=True, stop=True)
            hx = bp.tile([128, BLK], F32, tag="hx")
            nc.vector.tensor_scalar(out=hx[:, :nb], in0=hx_ps[:, :nb],
                                    scalar1=b1_, scalar2=None, op0=op.add)
            hy_ps = ps2.tile([128, BLK], F32, tag="mm")
            nc.tensor.matmul(out=hy_ps[:, :nb], lhsT=W1_, rhs=enc[:, :nb],
                             start=True, stop=True)
            hy = bp.tile([128, BLK], F32, tag="hy")
            nc.vector.tensor_scalar(out=hy[:, :nb], in0=hy_ps[:, :nb],
                                    scalar1=b1_, scalar2=None, op0=op.add)

            parts = sm.tile([128, 4], F32, tag="parts")
            nc.vector.tensor_reduce(out=parts[:, 0:1], in_=hx[:, :nb],
                                    axis=mybir.AxisListType.X, op=op.add)
            nc.vector.tensor_reduce(out=parts[:, 2:3], in_=hy[:, :nb],
                                    axis=mybir.AxisListType.X, op=op.add)
            scr = bp.tile([128, BLK], F32, tag="scr")
            nc.scalar.activation(out=scr[:, :nb], in_=hx[:, :nb],
                                 func=mybir.ActivationFunctionType.Square,
                                 bias=zeros_t[:],
                                 accum_out=parts[:, 1:2])
            scr2 = bp.tile([128, BLK], F32, tag="scr")
            nc.scalar.activation(out=scr2[:, :nb], in_=hy[:, :nb],
                                 func=mybir.ActivationFunctionType.Square,
                                 bias=zeros_t[:],
                                 accum_out=parts[:, 3:4])
            nc.vector.tensor_add(stats[:], stats[:], parts[:])

        # ---- stats AllReduce ----
        st_in = dram.tile([128, 4], F32)
        st_out = dram.tile([128, 4], F32)
        nc.gpsimd.dma_start(st_in[:], stats[:])
        nc.gpsimd.collective_compute(
            "AllReduce", op.add,
            replica_groups=[list(range(NCORES))],
            ins=[st_in[:].opt()],
            outs=[st_out[:].opt()],
        )
        stats_g = cst.tile([128, 4], F32, tag="stats_g")
        nc.gpsimd.dma_start(stats_g[:], st_out[:])

        # ---- BN coefficients: a = gamma/sqrt(var+eps), bb = beta - mu*a ----
        bn = cst.tile([128, 8], F32, tag="bn")  # mu_x ex2_x a_x bb_x mu_y ...
        coefs = []
        for pi, (c1, c2) in enumerate(((0, 1), (2, 3))):
            o = pi * 4
            mu = bn[:, o + 0 : o + 1]
            ex2 = bn[:, o + 1 : o + 2]
            a_c = bn[:, o + 2 : o + 3]
            bb_c = bn[:, o + 3 : o + 4]
            nc.vector.tensor_scalar(out=mu, in0=stats_g[:, c1 : c1 + 1],
                                    scalar1=1.0 / N, scalar2=None, op0=op.mult)
            nc.vector.tensor_scalar(out=ex2, in0=stats_g[:, c2 : c2 + 1],
                                    scalar1=1.0 / N, scalar2=None, op0=op.mult)
            musq = sm.tile([128, 1], F32, tag="t1")
            nc.vector.tensor_mul(musq[:], mu, mu)
            var = sm.tile([128, 1], F32, tag="t2")
            nc.vector.tensor_sub(var[:], ex2, musq[:])
            sd = sm.tile([128, 1], F32, tag="t3")
            nc.scalar.activation(out=sd[:], in_=var[:],
                                 func=mybir.ActivationFunctionType.Sqrt,
                                 bias=eps_)
            rs = sm.tile([128, 1], F32, tag="t4")
            nc.vector.reciprocal(rs[:], sd[:])
            nc.vector.tensor_mul(a_c, rs[:], gam_)
            t5 = sm.tile([128, 1], F32, tag="t5")
            nc.vector.tensor_mul(t5[:], mu, a_c)
            nc.vector.tensor_sub(bb_c, bet_, t5[:])
            coefs.append((a_c, bb_c))

        # ---- phase 2B: predictor + BYOL ----
        for b in range(NB):
            nb = min(BLK, RPC - b * BLK)
            cs = slice(b * BLK, b * BLK + nb)
            PT = aggPT[:, cs]
            QT = aggQT[:, cs]

            rt = bp.tile([128, BLK], F32, tag="rt")
            nc.vector.tensor_add(rt[:, :nb], PT, QT)

            # recompute onl/enc and hx/hy
            hcur = []
            for pi, rhs_src in enumerate((PT, None)):
                o_ps = ps2.tile([128, BLK], F32, tag="mm")
                rhs = rhs_src if rhs_src is not None else rt[:, :nb]
                nc.tensor.matmul(out=o_ps[:, :nb], lhsT=W_, rhs=rhs,
                                 start=True, stop=True)
                ob = bp.tile([128, BLK], F32, tag="onl")
                nc.vector.tensor_scalar(out=ob[:, :nb], in0=o_ps[:, :nb],
                                        scalar1=b_, scalar2=None, op0=op.add)
                h_ps = ps2.tile([128, BLK], F32, tag="mm")
                nc.tensor.matmul(out=h_ps[:, :nb], lhsT=W1_, rhs=ob[:, :nb],
                                 start=True, stop=True)
                a_c, bb_c = coefs[pi]
                # h = mm + b1 ; hn = h*a + bb  (fused: (mm+b1) then *a then +bb)
                hb = bp.tile([128, BLK], F32, tag="hx")
                nc.vector.tensor_scalar(out=hb[:, :nb], in0=h_ps[:, :nb],
                                        scalar1=b1_, scalar2=None, op0=op.add)
                hn = bp.tile([128, BLK], F32, tag="hy")
                nc.vector.tensor_scalar(out=hn[:, :nb], in0=hb[:, :nb],
                                        scalar1=a_c, scalar2=bb_c,
                                        op0=op.mult, op1=op.add)
                # prelu: max(hn,0) + pa*min(hn,0)
                hp = bp.tile([128, BLK], F32, tag="emb")
                nc.vector.tensor_scalar(out=hp[:, :nb], in0=hn[:, :nb],
                                        scalar1=0.0, scalar2=None, op0=op.max)
                neg = bp.tile([128, BLK], F32, tag="scr")
                nc.vector.tensor_scalar(out=neg[:, :nb], in0=hn[:, :nb],
                                        scalar1=0.0, scalar2=pa_,
                                        op0=op.min, op1=op.mult)
                nc.vector.tensor_add(hp[:, :nb], hp[:, :nb], neg[:, :nb])
                p_ps = ps2.tile([128, BLK], F32, tag="mm")
                nc.tensor.matmul(out=p_ps[:, :nb], lhsT=W2_, rhs=hp[:, :nb],
                                 start=True, stop=True)
                pb = bp.tile([128, BLK], F32, tag="x2b")
                nc.vector.tensor_scalar(out=pb[:, :nb], in0=p_ps[:, :nb],
                                        scalar1=b2_, scalar2=None, op0=op.add)
                # target for this path: pi=0 (px) pairs tgx=(P+Q)@Wt+bt;
                # pi=1 (py) pairs tgy=P@Wt+bt
                t_ps = ps2.tile([128, BLK], F32, tag="mm")
                t_rhs = rt[:, :nb] if pi == 0 else PT
                nc.tensor.matmul(out=t_ps[:, :nb], lhsT=Wt_, rhs=t_rhs,
                                 start=True, stop=True)
                tg = bp.tile([128, BLK], F32, tag="enc")
                nc.vector.tensor_scalar(out=tg[:, :nb], in0=t_ps[:, :nb],
                                        scalar1=bt_, scalar2=None, op0=op.add)
                hcur.append((pb, tg))

            for pi, (pb, tg) in enumerate(hcur):
                el = bp.tile([128, 3 * BLK], F32, tag="el")
                nc.vector.tensor_mul(el[:, 0:nb], pb[:, :nb], pb[:, :nb])
                nc.vector.tensor_mul(el[:, BLK : BLK + nb], tg[:, :nb],
                                     tg[:, :nb])
                nc.vector.tensor_mul(el[:, 2 * BLK : 2 * BLK + nb],
                                     pb[:, :nb], tg[:, :nb])
                cs_sb = byp.tile([1, 3 * BLK], F32, tag="cs_sb")
                for q in range(3):
                    cs_ps = psb.tile([1, BLK], F32)
                    nc.tensor.matmul(out=cs_ps[:, :nb], lhsT=ones_t[:],
                                     rhs=el[:, q * BLK : q * BLK + nb],
                                     start=True, stop=True)
                    nc.scalar.copy(cs_sb[:, q * BLK : q * BLK + nb],
                                   cs_ps[:, :nb])
                t1 = byp.tile([1, BLK], F32, tag="bt1")
                nc.vector.tensor_mul(t1[:, :nb], cs_sb[:, 0:nb],
                                     cs_sb[:, BLK : BLK + nb])
                t2 = byp.tile([1, BLK], F32, tag="bt2")
                nc.scalar.activation(out=t2[:, :nb], in_=t1[:, :nb],
                                     func=mybir.ActivationFunctionType.Sqrt,
                                     bias=zeros_t[0:1, :])
                nc.vector.reciprocal(t1[:, :nb], t2[:, :nb])
                nc.vector.tensor_mul(t2[:, :nb], cs_sb[:, 2 * BLK : 2 * BLK + nb],
                                     t1[:, :nb])
                nc.vector.tensor_add(dacc[:, :nb], dacc[:, :nb], t2[:, :nb])

        # ---- loss ----
        dtot = sm.tile([1, 8], F32, tag="dtot")
        nc.vector.memset(dtot[:], 0.0)
        nc.vector.tensor_reduce(out=dtot[:, 0:1], in_=dacc[:],
                                axis=mybir.AxisListType.X, op=op.add)
        l_in = dram.tile([1, 8], F32)
        l_out = dram.tile([1, 8], F32)
        nc.gpsimd.dma_start(l_in[:], dtot[:])
        nc.gpsimd.collective_compute(
            "AllReduce", op.add,
            replica_groups=[list(range(NCORES))],
            ins=[l_in[:].opt()],
            outs=[l_out[:].opt()],
        )
        dred = sm.tile([1, 8], F32, tag="dred")
        nc.gpsimd.dma_start(dred[:], l_out[:])
        lvec = sm.tile([1, 8], F32, tag="lvec")
        nc.vector.tensor_scalar(out=lvec[:], in0=dred[:],
                                scalar1=-2.0 / N, scalar2=4.0,
                                op0=op.mult, op1=op.add)
        nc.sync.dma_start(lossout[:], lvec[:])

    nc.compile()
    return nc


def kernel(x, perb, edge_row, edge_col, edge_val, W, b, Wt, bt,
           W1, b1, bn_gamma, bn_beta, prelu_a, W2, b2):
    x = np.asarray(x, dtype=np.float32)
    perb = np.asarray(perb, dtype=np.float32)
    edge_row = np.asarray(edge_row).astype(np.int64)
    edge_col = np.asarray(edge_col).astype(np.int64)
    edge_val = np.asarray(edge_val, dtype=np.float32)

    G = np.ascontiguousarray(np.concatenate([x, perb], axis=1), dtype=np.float32)
    g0, g1 = G[:HALF], G[HALF:]

    nch, totch, totidx, cores = _prep_edges(edge_row, edge_col, edge_val)

    x2 = x + perb
    wcat = np.ascontiguousarray(
        np.concatenate([W, Wt, W1, W2], axis=1), dtype=np.float32)
    prm = np.zeros((128, 8), dtype=np.float32)
    prm[:, 0] = b
    prm[:, 1] = bt
    prm[:, 2] = b1
    prm[:, 3] = b2
    prm[:, 4] = bn_gamma
    prm[:, 5] = bn_beta
    prm[:, 6] = float(prelu_a)
    prm[:, 7] = BN_EPS
    iota_np = np.tile(np.arange(128, dtype=np.float32)[None, :], (128, 1))

    in_maps = []
    for c in range(NCORES):
        x2t_c = np.ascontiguousarray(x2[c * RPC : (c + 1) * RPC].T)
        in_maps.append(dict(
            g0=g0, g1=g1,
            eidx=cores[c]["eidx"], dlv=cores[c]["dl"], vav=cores[c]["val"],
            x2t=x2t_c, wcat=wcat, prm=prm, iot=iota_np,
        ))

    nc = _build(nch, totch, totidx)
    res = run_bass_kernel_spmd(nc, in_maps, core_ids=list(range(NCORES)),
                               trace=TRACE, **TRACE_KW)
    LAST["res"] = res

    embed = np.concatenate(
        [res.results[c]["embedt"].T for c in range(NCORES)], axis=0)
    loss = np.float32(res.results[0]["lossout"][0, 0])
    return np.ascontiguousarray(embed, dtype=np.float32), loss


if __name__ == "__main__":
    import reference
    inputs = reference.setup_inputs()
    np_inputs = {k: np.asarray(v) for k, v in inputs.items()}
    emb, loss = kernel(**np_inputs)
    ref_emb, ref_loss = reference.reference(**inputs)
    ref_emb = np.asarray(ref_emb)
    e1 = np.abs(emb - ref_emb).max() / np.abs(ref_emb).max()
    e2 = abs(float(loss) - float(ref_loss)) / max(abs(float(ref_loss)), 1e-9)
    print(f"embed relerr {e1:.3e}  loss relerr {e2:.3e} "
          f"(loss {float(loss):.6f} vs {float(ref_loss):.6f})")
